# revision 12
# baseline (speedup 1.0000x reference)
"""Trainium2 Bass kernel for causal GQA self-attention (fused QKV + RoPE).

Problem: B=2, T=2048, C=2048, H=16 q-heads, KV=4 kv-heads, HD=128.
Sharding: 8 cores = (batch b, kv-group k). Each core computes the 4 q-heads
of one kv group for one batch element; outputs are disjoint slices of y.

Per-core schedule (all matmuls bf16, fp32 PSUM accumulate):
  prologue: project t-block 0 (two 3-bank passes), RoPE, V transpose.
  segments tt=1..3: attention for t-block tt-1 interleaved (via a unit
    scheduler) with the projection of t-block tt, so the PE array hides the
    ScalarE exp chain.
  tail: attention for t-block 3.
Attention is S^T-oriented, head-paired: one exp per (tb,sc) covers both
heads' [128,<=512] score chunks (3D AP over a 2-bank PSUM tile, bf16 out);
causal diagonal masked by triangular multiply; row sums via two CONCURRENT
col-tiled M=1 ones-matmuls (tile_position (0,0)/(0,32)); PV accumulates
unnormalized y^T in PSUM. Host divides by the sums and transposes.
"""

import math

import numpy as np

import concourse.bass as bass
import concourse.mybir as mybir
import concourse.tile as tile
from concourse import bacc
from concourse.bass_utils import run_bass_kernel_spmd

B, T, C = 2, 2048, 2048
H, KV, HD = 16, 4, 128
NREP = H // KV  # q heads per core
P = 128
NCORES = 8
CC_CHUNKS = C // P  # 16 contraction chunks
TT = 4  # t-blocks of 512
TB = T // TT  # 512
NB = 6  # j-blocks per core: q0..q3, k, v
SCALE = 1.0 / math.sqrt(HD)

f32 = mybir.dt.float32
bf16 = mybir.dt.bfloat16

TRACE = False  # set True (with ntff shim installed) to get exec_time_ns

_cache = {}


def _build():
    if "nc" in _cache:
        return _cache["nc"]

    nc = bacc.Bacc("TRN2", target_bir_lowering=False, debug=False,
                   num_devices=NCORES)

    xT_d = nc.dram_tensor("xT", [P, CC_CHUNKS, T], bf16, kind="ExternalInput").ap()
    wT_d = nc.dram_tensor("wT", [P, CC_CHUNKS, NB * P], bf16, kind="ExternalInput").ap()
    cc_d = nc.dram_tensor("CC", [P, T], bf16, kind="ExternalInput").ap()
    ss_d = nc.dram_tensor("SS2", [P, T], bf16, kind="ExternalInput").ap()
    tri_d = nc.dram_tensor("tri", [P, P], bf16, kind="ExternalInput").ap()
    ones_d = nc.dram_tensor("ones1", [P, 1], bf16, kind="ExternalInput").ap()
    ident_d = nc.dram_tensor("ident", [P, P], bf16, kind="ExternalInput").ap()
    # yT: [p, head, t] so DMA src/dst dims line up (host transposes)
    yT_d = nc.dram_tensor("yT", [P, NREP, T], bf16, kind="ExternalOutput").ap()
    sums_d = nc.dram_tensor("sums", [NREP, T], f32, kind="ExternalOutput").ap()

    with tile.TileContext(nc) as tc:
        with (
            tc.tile_pool(name="wt", bufs=1) as wt_pool,
            tc.tile_pool(name="xt", bufs=2) as xt_pool,
            tc.tile_pool(name="qkvt", bufs=1) as qkv_pool,
            tc.tile_pool(name="freq", bufs=1) as freq_pool,
            tc.tile_pool(name="small", bufs=1) as small_pool,
            tc.tile_pool(name="vsb", bufs=1) as v_pool,
            tc.tile_pool(name="swp", bufs=2) as swp_pool,
            tc.tile_pool(name="ropetmp", bufs=4) as rt_pool,
            tc.tile_pool(name="expt", bufs=3) as exp_pool,
            tc.tile_pool(name="yout", bufs=2) as y_pool,
            tc.tile_pool(name="psum", bufs=1, space="PSUM") as psum_pool,
        ):
            # ---- resident tensors ----
            tri = small_pool.tile([P, P], bf16, tag="tri", name="tri")
            ones1 = small_pool.tile([P, 1], bf16, tag="ones1", name="ones1")
            ident = small_pool.tile([P, P], bf16, tag="ident", name="ident")
            dummy = small_pool.tile([1, 2], f32, tag="dummy", name="dummy")
            wt_q = [wt_pool.tile([P, 4, NB * P], bf16, tag=f"wt{cq}",
                                 name=f"wt{cq}") for cq in range(4)]
            qkvT = qkv_pool.tile([P, NB, T], bf16, tag="qkvT", name="qkvT")
            v_sb = v_pool.tile([P, CC_CHUNKS, P], bf16, tag="vsb", name="v_sb")
            ccs = freq_pool.tile([P, T], bf16, tag="cc", name="ccs")
            ss2 = freq_pool.tile([P, T], bf16, tag="ss", name="ss2")
            ssb = y_pool.tile([97, T], f32, tag="ssb", name="ssb", bufs=1)

            # startup DMAs: finest chunks first so the PE starts ~10us in
            xts = {}
            xts[0] = xt_pool.tile([P, CC_CHUNKS, TB], bf16, tag="xt", name="xt0")
            for ci in range(4):
                nc.sync.dma_start(wt_q[0][:, ci, :], wT_d[:, ci, :])
                nc.sync.dma_start(xts[0][:, ci, :], xT_d[:, ci, 0:TB])
            nc.sync.dma_start(tri[:], tri_d[:])
            nc.sync.dma_start(ones1[:], ones_d[:])
            nc.sync.dma_start(ident[:], ident_d[:])
            nc.scalar.activation(dummy[:], tri[0:1, 0:2],
                                 mybir.ActivationFunctionType.Exp)
            for cq in range(1, 4):
                nc.sync.dma_start(wt_q[cq][:], wT_d[:, cq * 4:(cq + 1) * 4, :])
                nc.sync.dma_start(
                    xts[0][:, cq * 4:(cq + 1) * 4, :],
                    xT_d[:, cq * 4:(cq + 1) * 4, 0:TB])
            nc.sync.dma_start(ccs[:], cc_d[:])
            nc.sync.dma_start(ss2[:], ss_d[:])

            def fetch_x(tt):
                xts[tt] = xt_pool.tile([P, CC_CHUNKS, TB], bf16, tag="xt",
                                       name=f"xt{tt}")
                nc.sync.dma_start(xts[tt][:],
                                  xT_d[:, :, tt * TB:(tt + 1) * TB])

            fetch_x(1)

            # ---- projection units for one t-block ----
            # pass A: jb (k=4, v=5) -> T1a pair, q0 -> T3a single
            # pass B: jb (q1, q2) -> T1a pair, q3 -> T3a single
            def proj_units(tt):
                tsl = slice(tt * TB, (tt + 1) * TB)
                state = {}
                units = []
                # prologue (tt0): pass B borrows the attn tags (still free)
                # so its first matmuls don't WAR-wait on pass A's drain
                tags = {"A": ("T1a", "T3a"),
                        "B": ("T1b", "T3b") if tt == 0 else ("T1a", "T3a")}

                def alloc(pass_id):
                    tp, ts = tags[pass_id]
                    state[pass_id] = (
                        psum_pool.tile([P, 2, TB], f32, tag=tp,
                                       name=f"pp{tt}{pass_id}"),
                        psum_pool.tile([P, TB], f32, tag=ts,
                                       name=f"ps{tt}{pass_id}"),
                    )

                def mk_group(pass_id, cc, jbs):
                    def u():
                        if cc == 0 and pass_id == "A":
                            alloc("A")
                        if cc == 0 and pass_id == "B":
                            alloc("B")
                        pair, single = state[pass_id]
                        cq, ci = divmod(cc, 4)
                        for slot, jb in enumerate(jbs):
                            out = pair[:, slot, :] if slot < 2 else single[:]
                            nc.tensor.matmul(
                                out, wt_q[cq][:, ci, jb * P:(jb + 1) * P],
                                xts[tt][:, cc, :],
                                start=(cc == 0), stop=(cc == CC_CHUNKS - 1))
                    return u

                for cc in range(CC_CHUNKS):
                    units.append(mk_group("A", cc, (4, 5, 0)))

                def drain_a():
                    pair, single = state["A"]
                    nc.vector.tensor_copy(qkvT[:, 4:6, tsl], pair[:])
                    nc.vector.tensor_copy(qkvT[:, 0, tsl], single[:])
                units.append(drain_a)

                def mk_trp():
                    trp = psum_pool.tile([P, 2 * TB], bf16, tag="T3a",
                                         name=f"trp{tt}")
                    for i in range(4):
                        sc = 4 * tt + i
                        nc.tensor.transpose(
                            trp[:, i * P:(i + 1) * P],
                            qkvT[:, 5, sc * P:(sc + 1) * P], ident[:])
                        nc.vector.tensor_copy(v_sb[:, sc, :],
                                              trp[:, i * P:(i + 1) * P])
                units.append(mk_trp)

                def mk_rope(jb):
                    def u():
                        swp = swp_pool.tile([P, TB], bf16, tag="swp",
                                            name="swp")
                        nc.sync.dma_start(swp[0:64, :], qkvT[64:128, jb, tsl])
                        nc.sync.dma_start(swp[64:128, :], qkvT[0:64, jb, tsl])
                        ta = rt_pool.tile([P, TB], bf16, tag="rt", name="ta")
                        tb_ = rt_pool.tile([P, TB], bf16, tag="rt", name="tb")
                        nc.vector.tensor_tensor(
                            ta[:], qkvT[:, jb, tsl], ccs[:, tsl],
                            mybir.AluOpType.mult)
                        nc.vector.tensor_tensor(
                            tb_[:], swp[:], ss2[:, tsl], mybir.AluOpType.mult)
                        nc.vector.tensor_tensor(
                            qkvT[:, jb, tsl], ta[:], tb_[:],
                            mybir.AluOpType.add)
                    return u

                units.append(mk_rope(4))
                units.append(mk_rope(0))
                for cc in range(CC_CHUNKS):
                    units.append(mk_group("B", cc, (1, 2, 3)))

                def drain_b():
                    pair, single = state["B"]
                    nc.vector.tensor_copy(qkvT[:, 1:3, tsl], pair[:])
                    nc.vector.tensor_copy(qkvT[:, 3, tsl], single[:])
                units.append(drain_b)
                for jb in (1, 2, 3):
                    units.append(mk_rope(jb))
                return units

            # ---- attention for one t-block, with unit injection ----
            pending = []  # global pipeline: deferred PV/sums/drain closures
            s_tags = ["T2", "T1a"]
            s_idx = [0]

            def attn_tb(tb, units, tail):
                tsl = slice(tb * TB, (tb + 1) * TB)
                nsc = 4 * (tb + 1)
                n_iters = 2 * nsc
                done = [0]
                total = len(units)

                def inject(it):
                    # front-load: finish all units ~2 iters before block end
                    want = min(total, (total * (it + 3)) // n_iters)
                    while done[0] < want:
                        units[done[0]]()
                        done[0] += 1

                for hp in range(2):
                    ha, hb = 2 * hp, 2 * hp + 1
                    y_ps = psum_pool.tile([P, 2, TB], f32, tag="T1b",
                                          name=f"y{hp}_{tb}")
                    sum_ps = psum_pool.tile([P, TB], f32, tag="T3b",
                                            name=f"sm{hp}_{tb}")
                    for sc in range(nsc):
                        r = sc - 4 * tb
                        col0 = r * P if r >= 0 else 0
                        stag = s_tags[s_idx[0] % 2] if tail else "T2"
                        s_idx[0] += 1
                        s_ps = psum_pool.tile([P, 2, TB], f32, tag=stag,
                                              name=f"s{hp}_{tb}_{sc}")
                        kst = qkvT[:, 4, sc * P:(sc + 1) * P]
                        nc.tensor.matmul(
                            s_ps[:, 0, col0:], kst,
                            qkvT[:, ha, tb * TB + col0:(tb + 1) * TB],
                            start=True, stop=True)
                        nc.tensor.matmul(
                            s_ps[:, 1, col0:], kst,
                            qkvT[:, hb, tb * TB + col0:(tb + 1) * TB],
                            start=True, stop=True)
                        ex = exp_pool.tile([P, 2, TB], bf16, tag="expt",
                                           name=f"ex{hp}_{tb}_{sc}")
                        nc.scalar.activation(
                            ex[:, :, col0:], s_ps[:, :, col0:],
                            mybir.ActivationFunctionType.Exp, scale=SCALE)
                        if r >= 0:
                            for hi in (0, 1):
                                nc.vector.tensor_tensor(
                                    ex[:, hi, col0:col0 + P],
                                    ex[:, hi, col0:col0 + P],
                                    tri[:], mybir.AluOpType.mult)

                        def mk_flush(sc=sc, col0=col0, ex=ex, y_ps=y_ps,
                                     sum_ps=sum_ps, nsc=nsc, r0=64 * hp):
                            def f():
                                first, last = sc == 0, sc == nsc - 1
                                nc.tensor.matmul(
                                    y_ps[:, 0, col0:], v_sb[:, sc, :],
                                    ex[:, 0, col0:], start=first, stop=last)
                                nc.tensor.matmul(
                                    y_ps[:, 1, col0:], v_sb[:, sc, :],
                                    ex[:, 1, col0:], start=first, stop=last)
                                nc.tensor.matmul(
                                    sum_ps[r0:r0 + 1, col0:], ones1[:],
                                    ex[:, 0, col0:], start=first, stop=last,
                                    tile_position=(0, r0))
                                nc.tensor.matmul(
                                    sum_ps[r0 + 32:r0 + 33, col0:], ones1[:],
                                    ex[:, 1, col0:], start=first, stop=last,
                                    tile_position=(0, r0 + 32))
                            return f

                        pending.append(mk_flush())
                        if units:
                            inject(sc if hp == 0 else nsc + sc)
                        if len(pending) > 1:
                            pending.pop(0)()

                    def mk_drain(hp=hp, tb=tb, y_ps=y_ps, sum_ps=sum_ps,
                                 tsl=tsl, ha=ha, hb=hb, r0=64 * hp):
                        def f():
                            y_sb = y_pool.tile([P, 2, TB], bf16, tag="ysb",
                                               name="y_sb")
                            nc.vector.tensor_copy(y_sb[:], y_ps[:])
                            nc.sync.dma_start(
                                yT_d[:, 2 * hp:2 * hp + 2, tsl], y_sb[:])
                            nc.vector.tensor_copy(
                                ssb[r0:r0 + 33, tsl], sum_ps[r0:r0 + 33, :])
                            nc.sync.dma_start(sums_d[ha, tsl],
                                              ssb[r0:r0 + 1, tsl])
                            nc.sync.dma_start(sums_d[hb, tsl],
                                              ssb[r0 + 32:r0 + 33, tsl])
                        return f

                    pending.append(mk_drain())
                while len(pending) > 2:
                    pending.pop(0)()

            # ---- schedule ----
            for u in proj_units(0):
                u()
            for tt in range(1, TT):
                if tt + 1 < TT:
                    fetch_x(tt + 1)
                attn_tb(tt - 1, proj_units(tt), tail=False)
            attn_tb(TT - 1, [], tail=True)
            while pending:
                pending.pop(0)()

    nc.compile()
    _cache["nc"] = nc
    return nc


def _host_prep(x, w_qkv, freqs_cos, freqs_sin):
    """Build per-core input maps (numpy, cheap)."""
    import ml_dtypes
    bf = ml_dtypes.bfloat16
    x = np.asarray(x, dtype=np.float32)
    w_qkv = np.asarray(w_qkv, dtype=np.float32)
    freqs_cos = np.asarray(freqs_cos, dtype=np.float32)
    freqs_sin = np.asarray(freqs_sin, dtype=np.float32)

    perm = np.concatenate([np.arange(0, HD, 2), np.arange(1, HD, 2)])

    xTs = []
    for b in range(B):
        xt = np.ascontiguousarray(
            x[b].T.reshape(CC_CHUNKS, P, T).transpose(1, 0, 2)
        ).astype(bf)
        xTs.append(xt)

    cosT = freqs_cos.T  # [64, T]
    sinT = freqs_sin.T
    CCh = np.ascontiguousarray(np.concatenate([cosT, cosT], axis=0)).astype(bf)
    SS2 = np.ascontiguousarray(np.concatenate([-sinT, sinT], axis=0)).astype(bf)
    tri = np.triu(np.ones((P, P), dtype=np.float32)).astype(bf)
    ones1 = np.ones((P, 1), dtype=bf)
    ident = np.eye(P, dtype=np.float32).astype(bf)

    in_maps = []
    for core in range(NCORES):
        b, kv = divmod(core, KV)
        blocks = []
        for r in range(NREP):
            hrow = (kv * NREP + r) * HD
            blocks.append(w_qkv[hrow:hrow + HD][perm])
        blocks.append(w_qkv[H * HD + kv * HD:H * HD + (kv + 1) * HD][perm])
        blocks.append(
            w_qkv[(H + KV) * HD + kv * HD:(H + KV) * HD + (kv + 1) * HD]
        )
        w_shard = np.concatenate(blocks, axis=0)  # [768, C]
        wT = np.ascontiguousarray(
            w_shard.T.reshape(CC_CHUNKS, P, NB * P).transpose(1, 0, 2)
        ).astype(bf)
        in_maps.append({
            "xT": xTs[b],
            "wT": wT,
            "CC": CCh,
            "SS2": SS2,
            "tri": tri,
            "ones1": ones1,
            "ident": ident,
        })
    return in_maps


def kernel(x, w_qkv, freqs_cos, freqs_sin):
    nc = _build()
    in_maps = _host_prep(x, w_qkv, freqs_cos, freqs_sin)
    res = run_bass_kernel_spmd(nc, in_maps, list(range(NCORES)), trace=TRACE)
    _cache["last_res"] = res

    y = np.empty((B, T, C), dtype=np.float32)
    for core in range(NCORES):
        b, kv = divmod(core, KV)
        yT = np.asarray(res.results[core]["yT"], dtype=np.float32)
        yT = np.ascontiguousarray(yT.transpose(1, 0, 2))  # [NREP, 128, T]
        sums = res.results[core]["sums"]  # [NREP, T]
        yT = yT / sums[:, None, :]
        y[b, :, kv * NREP * HD:(kv + 1) * NREP * HD] = (
            yT.reshape(NREP * P, T).T
        )
    return y


# revision 14
# speedup vs baseline: 1.0278x; 1.0278x over previous
"""Trainium2 Bass kernel for causal GQA self-attention (fused QKV + RoPE).

Problem: B=2, T=2048, C=2048, H=16 q-heads, KV=4 kv-heads, HD=128.
Sharding: 8 cores = (batch b, kv-group k). Each core computes the 4 q-heads
of one kv group for one batch element; outputs are disjoint slices of y.

Per-core schedule (all matmuls bf16, fp32 PSUM accumulate):
  prologue: project t-block 0 (two 3-bank passes), RoPE, V transpose.
  segments tt=1..3: attention for t-block tt-1 interleaved (via a unit
    scheduler) with the projection of t-block tt, so the PE array hides the
    ScalarE exp chain.
  tail: attention for t-block 3.
Attention is S^T-oriented, head-paired: one exp per (tb,sc) covers both
heads' [128,<=512] score chunks (3D AP over a 2-bank PSUM tile, bf16 out);
causal diagonal masked by triangular multiply; row sums via two CONCURRENT
col-tiled M=1 ones-matmuls (tile_position (0,0)/(0,32)); PV accumulates
unnormalized y^T in PSUM. Host divides by the sums and transposes.
"""

import math

import numpy as np

import concourse.bass as bass
import concourse.mybir as mybir
import concourse.tile as tile
from concourse import bacc
from concourse.bass_utils import run_bass_kernel_spmd

B, T, C = 2, 2048, 2048
H, KV, HD = 16, 4, 128
NREP = H // KV  # q heads per core
P = 128
NCORES = 8
CC_CHUNKS = C // P  # 16 contraction chunks
TT = 4  # t-blocks of 512
TB = T // TT  # 512
NB = 6  # j-blocks per core: q0..q3, k, v
SCALE = 1.0 / math.sqrt(HD)

f32 = mybir.dt.float32
bf16 = mybir.dt.bfloat16

TRACE = False  # set True (with ntff shim installed) to get exec_time_ns

_cache = {}


def _build():
    if "nc" in _cache:
        return _cache["nc"]

    nc = bacc.Bacc("TRN2", target_bir_lowering=False, debug=False,
                   num_devices=NCORES)

    xT_d = nc.dram_tensor("xT", [P, CC_CHUNKS, T], bf16, kind="ExternalInput").ap()
    wT_d = nc.dram_tensor("wT", [P, CC_CHUNKS, NB * P], bf16, kind="ExternalInput").ap()
    cc_d = nc.dram_tensor("CC", [P, T], bf16, kind="ExternalInput").ap()
    ss_d = nc.dram_tensor("SS2", [P, T], bf16, kind="ExternalInput").ap()
    tri_d = nc.dram_tensor("tri", [P, P], bf16, kind="ExternalInput").ap()
    ones_d = nc.dram_tensor("ones1", [P, 1], bf16, kind="ExternalInput").ap()
    ident_d = nc.dram_tensor("ident", [P, P], bf16, kind="ExternalInput").ap()
    # yT: [p, head, t] so DMA src/dst dims line up (host transposes)
    yT_d = nc.dram_tensor("yT", [P, NREP, T], bf16, kind="ExternalOutput").ap()
    sums_d = nc.dram_tensor("sums", [NREP, T], f32, kind="ExternalOutput").ap()

    with tile.TileContext(nc) as tc:
        with (
            tc.tile_pool(name="wt", bufs=1) as wt_pool,
            tc.tile_pool(name="xt", bufs=2) as xt_pool,
            tc.tile_pool(name="qkvt", bufs=1) as qkv_pool,
            tc.tile_pool(name="freq", bufs=1) as freq_pool,
            tc.tile_pool(name="small", bufs=1) as small_pool,
            tc.tile_pool(name="vsb", bufs=1) as v_pool,
            tc.tile_pool(name="swp", bufs=2) as swp_pool,
            tc.tile_pool(name="ropetmp", bufs=4) as rt_pool,
            tc.tile_pool(name="expt", bufs=3) as exp_pool,
            tc.tile_pool(name="yout", bufs=2) as y_pool,
            tc.tile_pool(name="psum", bufs=1, space="PSUM") as psum_pool,
        ):
            # ---- resident tensors ----
            tri = small_pool.tile([P, P], bf16, tag="tri", name="tri")
            ones1 = small_pool.tile([P, 1], bf16, tag="ones1", name="ones1")
            ident = small_pool.tile([P, P], bf16, tag="ident", name="ident")
            dummy = small_pool.tile([1, 2], f32, tag="dummy", name="dummy")
            wt_q = [wt_pool.tile([P, 4, NB * P], bf16, tag=f"wt{cq}",
                                 name=f"wt{cq}") for cq in range(4)]
            qkvT = qkv_pool.tile([P, NB, T], bf16, tag="qkvT", name="qkvT")
            v_sb = v_pool.tile([P, CC_CHUNKS, P], bf16, tag="vsb", name="v_sb")
            ccs = freq_pool.tile([P, T], bf16, tag="cc", name="ccs")
            ss2 = freq_pool.tile([P, T], bf16, tag="ss", name="ss2")
            ssb = y_pool.tile([97, T], f32, tag="ssb", name="ssb", bufs=1)

            # startup DMAs: w0 + first x quarter up front, consts after,
            # then alternate w/x so arrival stays ahead of consumption
            xts = {}
            xts[0] = xt_pool.tile([P, CC_CHUNKS, TB], bf16, tag="xt", name="xt0")
            nc.sync.dma_start(wt_q[0][:], wT_d[:, 0:4, :])
            nc.sync.dma_start(xts[0][:, 0:4, :], xT_d[:, 0:4, 0:TB])
            nc.sync.dma_start(tri[:], tri_d[:])
            nc.sync.dma_start(ones1[:], ones_d[:])
            nc.sync.dma_start(ident[:], ident_d[:])
            nc.scalar.activation(dummy[:], tri[0:1, 0:2],
                                 mybir.ActivationFunctionType.Exp)
            for cq in range(1, 4):
                nc.sync.dma_start(wt_q[cq][:], wT_d[:, cq * 4:(cq + 1) * 4, :])
                nc.sync.dma_start(
                    xts[0][:, cq * 4:(cq + 1) * 4, :],
                    xT_d[:, cq * 4:(cq + 1) * 4, 0:TB])
            nc.sync.dma_start(ccs[:], cc_d[:])
            nc.sync.dma_start(ss2[:], ss_d[:])

            def fetch_x(tt):
                xts[tt] = xt_pool.tile([P, CC_CHUNKS, TB], bf16, tag="xt",
                                       name=f"xt{tt}")
                nc.sync.dma_start(xts[tt][:],
                                  xT_d[:, :, tt * TB:(tt + 1) * TB])

            fetch_x(1)

            # ---- projection units for one t-block ----
            # pass A: jb (k=4, v=5) -> T1a pair, q0 -> T3a single
            # pass B: jb (q1, q2) -> T1a pair, q3 -> T3a single
            def proj_units(tt):
                tsl = slice(tt * TB, (tt + 1) * TB)
                state = {}
                units = []
                # prologue (tt0): pass B borrows the attn tags (still free)
                # so its first matmuls don't WAR-wait on pass A's drain
                tags = {"A": ("T1a", "T3a"),
                        "B": ("T1b", "T3b") if tt == 0 else ("T1a", "T3a")}

                def alloc(pass_id):
                    tp, ts = tags[pass_id]
                    state[pass_id] = (
                        psum_pool.tile([P, 2, TB], f32, tag=tp,
                                       name=f"pp{tt}{pass_id}"),
                        psum_pool.tile([P, TB], f32, tag=ts,
                                       name=f"ps{tt}{pass_id}"),
                    )

                def mk_group(pass_id, cc, jbs):
                    def u():
                        if cc == 0 and pass_id == "A":
                            alloc("A")
                        if cc == 0 and pass_id == "B":
                            alloc("B")
                        pair, single = state[pass_id]
                        cq, ci = divmod(cc, 4)
                        for slot, jb in enumerate(jbs):
                            out = pair[:, slot, :] if slot < 2 else single[:]
                            nc.tensor.matmul(
                                out, wt_q[cq][:, ci, jb * P:(jb + 1) * P],
                                xts[tt][:, cc, :],
                                start=(cc == 0), stop=(cc == CC_CHUNKS - 1))
                    return u

                for cc in range(CC_CHUNKS):
                    units.append(mk_group("A", cc, (4, 5, 0)))

                def drain_a():
                    pair, single = state["A"]
                    nc.vector.tensor_copy(qkvT[:, 4:6, tsl], pair[:])
                    nc.vector.tensor_copy(qkvT[:, 0, tsl], single[:])
                units.append(drain_a)

                def mk_trp():
                    trp = psum_pool.tile([P, 2 * TB], bf16, tag="T3a",
                                         name=f"trp{tt}")
                    for i in range(4):
                        sc = 4 * tt + i
                        nc.tensor.transpose(
                            trp[:, i * P:(i + 1) * P],
                            qkvT[:, 5, sc * P:(sc + 1) * P], ident[:])
                        nc.vector.tensor_copy(v_sb[:, sc, :],
                                              trp[:, i * P:(i + 1) * P])
                units.append(mk_trp)

                def mk_rope(jb):
                    def u():
                        swp = swp_pool.tile([P, TB], bf16, tag="swp",
                                            name="swp")
                        nc.sync.dma_start(swp[0:64, :], qkvT[64:128, jb, tsl])
                        nc.sync.dma_start(swp[64:128, :], qkvT[0:64, jb, tsl])
                        ta = rt_pool.tile([P, TB], bf16, tag="rt", name="ta")
                        tb_ = rt_pool.tile([P, TB], bf16, tag="rt", name="tb")
                        nc.vector.tensor_tensor(
                            ta[:], qkvT[:, jb, tsl], ccs[:, tsl],
                            mybir.AluOpType.mult)
                        nc.vector.tensor_tensor(
                            tb_[:], swp[:], ss2[:, tsl], mybir.AluOpType.mult)
                        nc.vector.tensor_tensor(
                            qkvT[:, jb, tsl], ta[:], tb_[:],
                            mybir.AluOpType.add)
                    return u

                units.append(mk_rope(4))
                units.append(mk_rope(0))
                for cc in range(CC_CHUNKS):
                    units.append(mk_group("B", cc, (1, 2, 3)))

                def drain_b():
                    pair, single = state["B"]
                    nc.vector.tensor_copy(qkvT[:, 1:3, tsl], pair[:])
                    nc.vector.tensor_copy(qkvT[:, 3, tsl], single[:])
                units.append(drain_b)
                for jb in (1, 2, 3):
                    units.append(mk_rope(jb))
                return units

            # ---- attention for one t-block, with unit injection ----
            pending = []  # global pipeline: deferred PV/sums/drain closures
            s_tags = ["T2", "T1a"]
            s_idx = [0]

            def attn_tb(tb, units, tail):
                tsl = slice(tb * TB, (tb + 1) * TB)
                nsc = 4 * (tb + 1)
                n_iters = 2 * nsc
                done = [0]
                total = len(units)

                def inject(it):
                    # front-load: finish all units ~2 iters before block end
                    want = min(total, (total * (it + 3)) // n_iters)
                    while done[0] < want:
                        units[done[0]]()
                        done[0] += 1

                for hp in range(2):
                    ha, hb = 2 * hp, 2 * hp + 1
                    y_ps = psum_pool.tile([P, 2, TB], f32, tag="T1b",
                                          name=f"y{hp}_{tb}")
                    sum_ps = psum_pool.tile([P, TB], f32, tag="T3b",
                                            name=f"sm{hp}_{tb}")
                    for sc in range(nsc):
                        r = sc - 4 * tb
                        col0 = r * P if r >= 0 else 0
                        stag = s_tags[s_idx[0] % 2] if tail else "T2"
                        s_idx[0] += 1
                        s_ps = psum_pool.tile([P, 2, TB], f32, tag=stag,
                                              name=f"s{hp}_{tb}_{sc}")
                        kst = qkvT[:, 4, sc * P:(sc + 1) * P]
                        nc.tensor.matmul(
                            s_ps[:, 0, col0:], kst,
                            qkvT[:, ha, tb * TB + col0:(tb + 1) * TB],
                            start=True, stop=True)
                        nc.tensor.matmul(
                            s_ps[:, 1, col0:], kst,
                            qkvT[:, hb, tb * TB + col0:(tb + 1) * TB],
                            start=True, stop=True)
                        ex = exp_pool.tile([P, 2, TB], bf16, tag="expt",
                                           name=f"ex{hp}_{tb}_{sc}")
                        nc.scalar.activation(
                            ex[:, :, col0:], s_ps[:, :, col0:],
                            mybir.ActivationFunctionType.Exp, scale=SCALE)
                        if r >= 0:
                            for hi in (0, 1):
                                nc.vector.tensor_tensor(
                                    ex[:, hi, col0:col0 + P],
                                    ex[:, hi, col0:col0 + P],
                                    tri[:], mybir.AluOpType.mult)

                        def mk_flush(sc=sc, col0=col0, ex=ex, y_ps=y_ps,
                                     sum_ps=sum_ps, nsc=nsc, r0=64 * hp):
                            def f():
                                first, last = sc == 0, sc == nsc - 1
                                nc.tensor.matmul(
                                    y_ps[:, 0, col0:], v_sb[:, sc, :],
                                    ex[:, 0, col0:], start=first, stop=last)
                                nc.tensor.matmul(
                                    y_ps[:, 1, col0:], v_sb[:, sc, :],
                                    ex[:, 1, col0:], start=first, stop=last)
                                nc.tensor.matmul(
                                    sum_ps[r0:r0 + 1, col0:], ones1[:],
                                    ex[:, 0, col0:], start=first, stop=last,
                                    tile_position=(0, r0))
                                nc.tensor.matmul(
                                    sum_ps[r0 + 32:r0 + 33, col0:], ones1[:],
                                    ex[:, 1, col0:], start=first, stop=last,
                                    tile_position=(0, r0 + 32))
                            return f

                        pending.append(mk_flush())
                        if units:
                            inject(sc if hp == 0 else nsc + sc)
                        if len(pending) > 1:
                            pending.pop(0)()

                    def mk_drain(hp=hp, tb=tb, y_ps=y_ps, sum_ps=sum_ps,
                                 tsl=tsl, ha=ha, hb=hb, r0=64 * hp):
                        def f():
                            y_sb = y_pool.tile([P, 2, TB], bf16, tag="ysb",
                                               name="y_sb")
                            nc.vector.tensor_copy(y_sb[:], y_ps[:])
                            nc.sync.dma_start(
                                yT_d[:, 2 * hp:2 * hp + 2, tsl], y_sb[:])
                            nc.vector.tensor_copy(
                                ssb[r0:r0 + 33, tsl], sum_ps[r0:r0 + 33, :])
                            if tb == TT - 1:
                                nc.sync.dma_start(sums_d[ha, :],
                                                  ssb[r0:r0 + 1, :])
                                nc.sync.dma_start(sums_d[hb, :],
                                                  ssb[r0 + 32:r0 + 33, :])
                        return f

                    pending.append(mk_drain())
                while len(pending) > 2:
                    pending.pop(0)()

            # ---- schedule ----
            for u in proj_units(0):
                u()
            for tt in range(1, TT):
                if tt + 1 < TT:
                    fetch_x(tt + 1)
                attn_tb(tt - 1, proj_units(tt), tail=False)
            attn_tb(TT - 1, [], tail=True)
            while pending:
                pending.pop(0)()

    nc.compile()
    _cache["nc"] = nc
    return nc


def _host_prep(x, w_qkv, freqs_cos, freqs_sin):
    """Build per-core input maps (numpy, cheap)."""
    import ml_dtypes
    bf = ml_dtypes.bfloat16
    x = np.asarray(x, dtype=np.float32)
    w_qkv = np.asarray(w_qkv, dtype=np.float32)
    freqs_cos = np.asarray(freqs_cos, dtype=np.float32)
    freqs_sin = np.asarray(freqs_sin, dtype=np.float32)

    perm = np.concatenate([np.arange(0, HD, 2), np.arange(1, HD, 2)])

    xTs = []
    for b in range(B):
        xt = np.ascontiguousarray(
            x[b].T.reshape(CC_CHUNKS, P, T).transpose(1, 0, 2)
        ).astype(bf)
        xTs.append(xt)

    cosT = freqs_cos.T  # [64, T]
    sinT = freqs_sin.T
    CCh = np.ascontiguousarray(np.concatenate([cosT, cosT], axis=0)).astype(bf)
    SS2 = np.ascontiguousarray(np.concatenate([-sinT, sinT], axis=0)).astype(bf)
    tri = np.triu(np.ones((P, P), dtype=np.float32)).astype(bf)
    ones1 = np.ones((P, 1), dtype=bf)
    ident = np.eye(P, dtype=np.float32).astype(bf)

    in_maps = []
    for core in range(NCORES):
        b, kv = divmod(core, KV)
        blocks = []
        for r in range(NREP):
            hrow = (kv * NREP + r) * HD
            blocks.append(w_qkv[hrow:hrow + HD][perm])
        blocks.append(w_qkv[H * HD + kv * HD:H * HD + (kv + 1) * HD][perm])
        blocks.append(
            w_qkv[(H + KV) * HD + kv * HD:(H + KV) * HD + (kv + 1) * HD]
        )
        w_shard = np.concatenate(blocks, axis=0)  # [768, C]
        wT = np.ascontiguousarray(
            w_shard.T.reshape(CC_CHUNKS, P, NB * P).transpose(1, 0, 2)
        ).astype(bf)
        in_maps.append({
            "xT": xTs[b],
            "wT": wT,
            "CC": CCh,
            "SS2": SS2,
            "tri": tri,
            "ones1": ones1,
            "ident": ident,
        })
    return in_maps


def kernel(x, w_qkv, freqs_cos, freqs_sin):
    nc = _build()
    in_maps = _host_prep(x, w_qkv, freqs_cos, freqs_sin)
    res = run_bass_kernel_spmd(nc, in_maps, list(range(NCORES)), trace=TRACE)
    _cache["last_res"] = res

    y = np.empty((B, T, C), dtype=np.float32)
    for core in range(NCORES):
        b, kv = divmod(core, KV)
        yT = np.asarray(res.results[core]["yT"], dtype=np.float32)
        yT = np.ascontiguousarray(yT.transpose(1, 0, 2))  # [NREP, 128, T]
        sums = res.results[core]["sums"]  # [NREP, T]
        yT = yT / sums[:, None, :]
        y[b, :, kv * NREP * HD:(kv + 1) * NREP * HD] = (
            yT.reshape(NREP * P, T).T
        )
    return y
